# revision 1
# baseline (speedup 1.0000x reference)
"""Trainium2 Bass kernel for nn_AcceptHead: out = fc2(gelu(fc1(LN(x)))).

Self-contained: hardcodes shapes (B=4, L=4096, H=4096, F=1024) and the
data-parallel sharding (tokens split 8 ways, head params replicated).

"LN-fold" architecture: the device PE stream is *only* fc1 matmuls.
LayerNorm is folded into the matmul epilogue:

    LN(x)@W = r_t * (x@W) - r_t * mu_t * colsum(W)     (gamma folded into W)

  - x arrives already TRANSPOSED from the host (xts: [128, chunk, k, tok]
    fp16, 256-token chunk-blocked) -- no on-device transpose/normalize.
  - fc1: lhsT = xT block [128h x 128t], rhs = w1ts [128h x 512f], PSUM
    accumulation over 32 k-tiles (fp16 -- the PE floor, ~218us @2.4GHz).
  - mean correction: one K=1 matmul per (t-tile, f-half) adds
    (-mu_t) * colsum_f into the same PSUM group (lhsT = row of -mu at
    partition 0 made by a tiny PE transpose; rhs = colsum row).
  - rstd r_t is applied as the per-partition `scale` AP of the Gelu
    activation: g = Gelu(r * psum). Newton rsqrt on DVE (bit-trick seed).
  - stats (sum via DVE reduce / sumsq via ACT Square accum_out) come from
    a second wire copy of x in [tok, H] fp8 layout; stats are emitted a
    full chunk ahead so the in-order DVE queue never blocks the PE.
  - fc2 as DVE dot (g * w2_bcast, reduce); out written [tile, 128] via a
    final PE transpose so the out DMA is 16 contiguous rows.

Schedule notes (measured on HW via ntff traces; all load-bearing):
  - DMA bandwidth is SHARED (~350 GB/s/core across all queues), so the
    fill is bandwidth-bound: stage 1 = w1ts f-half 0 (SP lo-k / SWDGE
    hi-k) + chunk-0 x on the ACT ring, stage 2 = f-half 1 (ACT lo-k /
    SP hi-k), stage 3 = xs-c0 + later chunks on SP. k is consumed
    lo/hi-interleaved (k0,k16,k1,...) matching the queue split so a slow
    queue half doesn't stall the chase.
  - Any PE idle gap triggers a HAM re-throttle (K=8->4 rows, ~7us at
    half speed), so gap avoidance pays double. This exact schedule runs
    K=8 continuously through the steady state.
  - per-(tile,fh) epilogues interleave right behind each tile's k-loops;
    -mu rows are emitted at the top of each chunk body (stats ran a
    chunk earlier) so their DVE copy retires instantly.
  - 256-token xts chunk blocks keep DMA inner runs at 512B; smaller
    blocks measurably degrade aggregate DMA bandwidth.

fp8 matmul was investigated and rejected: DoubleRow measures 2x fp16 per
unit contraction on this HW (3826ns vs 7386ns per K=4096,N=512 group;
the cost model's 0.5 cyc/row = 4x is wrong here), and at 2x every
precision-passing fp8 scheme costs the same as fp16 (1-pass e4m3 fails
the 2e-2 gate at 4.2e-2 measured in simulation).
"""

import os
import sys

for _p in ("/opt/trn_rl_repo", "/root/.axon_site/_ro/trn_rl_repo"):
    if os.path.isdir(_p) and _p not in sys.path:
        sys.path.append(_p)

import numpy as np

import concourse.bacc as bacc
import concourse.mybir as mybir
import concourse.tile as tile
from concourse.bass_utils import run_bass_kernel_spmd

N_CORES = 8
B, L, H = 4, 4096, 4096
F = H // 4
F2 = F // 2                   # 512, f-half width
T_TOT = B * L                 # 16384 tokens
T_CORE = T_TOT // N_CORES     # 2048 tokens per core
P = 128
KT = H // P                   # 32 contraction tiles
CHUNK_T = 256                 # tokens per pipeline chunk
N_CHUNKS = T_CORE // CHUNK_T  # 8
TT = CHUNK_T // P             # t-tiles per chunk (2)
N_TTILES = T_CORE // P        # 16
EPS = 1e-5
RSQRT_MAGIC = 0x5F3759DF

F16 = mybir.dt.float16
F32 = mybir.dt.float32
F8 = mybir.dt.float8e4
I32 = mybir.dt.int32
AF = mybir.ActivationFunctionType
ALU = mybir.AluOpType


def build_program(has_bias1: bool, bias2_val: float):
    nc = bacc.Bacc(
        "TRN2",
        target_bir_lowering=False,
        debug=False,
        enable_asserts=False,
        num_devices=N_CORES,
    )
    # x, transposed+chunk-blocked on host: xts[p, c, k, t] = x[c*256+t, k*128+p]
    xts_d = nc.dram_tensor(
        "xts", [P, N_CHUNKS, KT, CHUNK_T], F16, kind="ExternalInput"
    ).ap()
    # x, natural [tok, H] layout (stats only; fp8 halves its wire cost and
    # the ~2.6% quantization only perturbs mu/r by ~4e-4 relative)
    xs_d = nc.dram_tensor("xs", [T_CORE, H], F8, kind="ExternalInput").ap()
    # w1 (gamma-folded, transposed): w1ts[p, fh, k, j] = w1g[k*128+p, fh*512+j]
    w1ts_d = nc.dram_tensor(
        "w1ts", [P, 2, KT, F2], F16, kind="ExternalInput"
    ).ap()
    cs_d = nc.dram_tensor("cs", [1, F], F16, kind="ExternalInput").ap()
    w2b_d = nc.dram_tensor("w2b", [P, F], F16, kind="ExternalInput").ap()
    if has_bias1:
        b1r_d = nc.dram_tensor("b1r", [1, F], F16, kind="ExternalInput").ap()
    # out as [t-tile, partition]: token t = n*128+p lives at out[n, p], so
    # the flattened DRAM tensor IS token order. A direct (n p) -> p n
    # scatter DMA would cost ~10us in 4-byte descriptors.
    out_d = nc.dram_tensor(
        "out", [N_TTILES, P], F32, kind="ExternalOutput"
    ).ap()

    with tile.TileContext(nc) as tc:
        with (
            tc.tile_pool(name="singles", bufs=1) as singles,
            tc.tile_pool(name="xtpool", bufs=3) as xtpool,
            tc.tile_pool(name="xspool", bufs=6) as xspool,
            tc.tile_pool(name="sqscr", bufs=1) as sqscr_pool,
            tc.tile_pool(name="gpool", bufs=2) as gpool,
            tc.tile_pool(name="fc2scr", bufs=1) as fc2scr_pool,
            tc.tile_pool(name="stats", bufs=4) as stats,
            tc.tile_pool(name="nrow", bufs=2) as nrow_pool,
            tc.tile_pool(name="psum", bufs=3, space="PSUM") as psum_pool,
            tc.tile_pool(name="tpsum", bufs=1, space="PSUM") as tpsum_pool,
            tc.tile_pool(name="opsum", bufs=1, space="PSUM") as opsum_pool,
        ):
            # ---- fill-phase DMA schedule (see module docstring) ----
            w1ts_sb = singles.tile([P, 2, KT, F2], F16)

            def w1_dma(eng, fh, ka, kb):
                eng.dma_start(
                    out=w1ts_sb[:, fh, ka:kb, :], in_=w1ts_d[:, fh, ka:kb, :]
                )

            # stage 1: ACT ring carries xts-c0; SP + SWDGE carry fh0.
            xt0 = xtpool.tile([P, KT, CHUNK_T], F16, tag="xt")
            for ka in range(0, KT // 2, 4):
                for k0 in (ka, KT // 2 + ka):  # lo/hi interleave, see k-loop
                    nc.scalar.dma_start(
                        out=xt0[:, k0 : k0 + 4, :],
                        in_=xts_d[:, 0, k0 : k0 + 4, :],
                    )
            for k0 in range(0, KT // 2, 4):
                w1_dma(nc.sync, 0, k0, k0 + 4)
            for k0 in range(KT // 2, KT, 4):
                w1_dma(nc.gpsimd, 0, k0, k0 + 4)
            # stage 2: fh1 split ACT/SP
            for k0 in range(0, KT // 2, 4):
                w1_dma(nc.scalar, 1, k0, k0 + 4)
            for k0 in range(KT // 2, KT, 4):
                w1_dma(nc.sync, 1, k0, k0 + 4)
            # stage 3: xs-c0 on ACT, small params on SWDGE
            xss0 = []
            for i in range(TT):
                xs = xspool.tile([P, H], F8, tag="xs")
                nc.scalar.dma_start(out=xs, in_=xs_d[i * P : (i + 1) * P, :])
                xss0.append(xs)
            cs_sb = singles.tile([1, F], F16)
            nc.gpsimd.dma_start(out=cs_sb, in_=cs_d)
            w2b_sb = singles.tile([P, F], F16)
            nc.gpsimd.dma_start(out=w2b_sb, in_=w2b_d)
            if has_bias1:
                b1r_sb = singles.tile([1, F], F16)
                nc.gpsimd.dma_start(out=b1r_sb, in_=b1r_d)
            outcols = singles.tile([P, N_TTILES], F32)
            outrow = singles.tile([N_TTILES, P], F32)
            ident = singles.tile([P, P], F16)
            ident32 = singles.tile([P, P], F32)
            from concourse.masks import make_identity
            make_identity(nc, ident[:])
            make_identity(nc, ident32[:])

            # ---- chunk input loads for chunks 1+ (SP ring) ----
            def load_chunk(c):
                xt = xtpool.tile([P, KT, CHUNK_T], F16, tag="xt")
                for ka in range(0, KT // 2, 4):
                    for k0 in (ka, KT // 2 + ka):
                        nc.sync.dma_start(
                            out=xt[:, k0 : k0 + 4, :],
                            in_=xts_d[:, c, k0 : k0 + 4, :],
                        )
                xss = []
                for i in range(TT):
                    xs = xspool.tile([P, H], F8, tag="xs")
                    row0 = c * CHUNK_T + i * P
                    nc.sync.dma_start(out=xs, in_=xs_d[row0 : row0 + P, :])
                    xss.append(xs)
                return xt, xss

            # ---- stats chain (DVE + ACT), emitted one chunk AHEAD of the
            # chunk's k-loops so the in-order DVE queue never makes a corr
            # matmul wait behind the previous chunk's fc2 work. ----
            def emit_stats(xss):
                sums = stats.tile([P, TT], F32, tag="sums")
                sq = stats.tile([P, TT], F32, tag="sq")
                for i in range(TT):
                    nc.vector.reduce_sum(
                        sums[:, i : i + 1], xss[i], axis=mybir.AxisListType.X
                    )
                    sqs = sqscr_pool.tile([P, H], F16, tag="sqs")
                    nc.scalar.activation(
                        out=sqs, in_=xss[i], func=AF.Square,
                        accum_out=sq[:, i : i + 1],
                    )
                mu = stats.tile([P, TT], F32, tag="mu")
                nc.vector.tensor_scalar_mul(mu, sums, 1.0 / H)
                vv = stats.tile([P, TT], F32, tag="vv")
                # vv = sq/H - mu^2 + eps
                nc.vector.tensor_tensor(out=vv, in0=mu, in1=mu, op=ALU.mult)
                nc.vector.tensor_scalar(
                    out=vv, in0=vv, scalar1=-1.0, scalar2=EPS,
                    op0=ALU.mult, op1=ALU.add,
                )
                nc.vector.tensor_scalar(
                    out=sq, in0=sq, scalar1=1.0 / H, scalar2=None, op0=ALU.mult
                )
                nc.vector.tensor_tensor(out=vv, in0=vv, in1=sq, op=ALU.add)
                # Newton rsqrt: y0 via bit trick, 2 iterations
                y = stats.tile([P, TT], F32, tag="y")
                yi = y[:].bitcast(I32)
                nc.vector.tensor_scalar(
                    out=yi, in0=vv[:].bitcast(I32), scalar1=1, scalar2=None,
                    op0=ALU.arith_shift_right,
                )
                nc.vector.tensor_scalar(
                    out=yi, in0=yi, scalar1=-1, scalar2=RSQRT_MAGIC,
                    op0=ALU.mult, op1=ALU.add,
                )
                h_half = stats.tile([P, TT], F32, tag="h_half")
                nc.vector.tensor_scalar_mul(h_half, vv, 0.5)
                u = stats.tile([P, TT], F32, tag="u")
                for _ in range(2):
                    nc.vector.tensor_tensor(out=u, in0=y, in1=y, op=ALU.mult)
                    nc.vector.tensor_tensor(out=u, in0=u, in1=h_half, op=ALU.mult)
                    nc.vector.tensor_scalar(
                        out=u, in0=u, scalar1=-1.0, scalar2=1.5,
                        op0=ALU.mult, op1=ALU.add,
                    )
                    nc.vector.tensor_tensor(out=y, in0=y, in1=u, op=ALU.mult)
                # nmr16 = -mu as fp16 (the corr-matmul lhsT operand)
                nmr16 = stats.tile([P, TT], F16, tag="nmr16")
                nc.vector.tensor_scalar_mul(nmr16, mu, -1.0)
                if has_bias1:
                    # invr = sqrt(var+eps) = vv * y; bias row b1_eff enters
                    # PSUM as invr_row.T @ b1r so that Gelu's r-scale cancels.
                    invr16 = stats.tile([P, TT], F16, tag="invr16")
                    nc.vector.tensor_tensor(
                        out=invr16, in0=vv, in1=y, op=ALU.mult
                    )
                else:
                    invr16 = None
                return y, nmr16, invr16

            cur = (xt0, xss0)
            nxt = load_chunk(1)
            st_cur = emit_stats(cur[1])

            nr = ir = None
            for c in range(N_CHUNKS):
                xt, xss = cur
                cur = nxt
                y, nmr16, invr16 = st_cur

                # ---- -mu rows to partition 0 via PE transpose; top of
                # chunk body for c>=1 (nmr16 computed a full chunk ago) so
                # the PE transpose + DVE copy retire immediately. ----
                def emit_nmr_rows():
                    tps = tpsum_pool.tile([1, TT, P], F16, tag="tps")
                    for i in range(TT):
                        nc.tensor.transpose(
                            tps[:, i, :], nmr16[:, i : i + 1], ident[:]
                        )
                    nr = nrow_pool.tile([1, TT, P], F16, tag="nr")
                    nc.vector.tensor_copy(out=nr, in_=tps)
                    if has_bias1:
                        tps2 = tpsum_pool.tile([1, TT, P], F16, tag="tps")
                        for i in range(TT):
                            nc.tensor.transpose(
                                tps2[:, i, :], invr16[:, i : i + 1], ident[:]
                            )
                        ir = nrow_pool.tile([1, TT, P], F16, tag="ir")
                        nc.vector.tensor_copy(out=ir, in_=tps2)
                    else:
                        ir = None
                    return nr, ir

                if c > 0:
                    nr, ir = emit_nmr_rows()
                if c + 1 < N_CHUNKS:
                    st_cur = emit_stats(cur[1])

                def emit_group(i, fh, g_ps):
                    fcols = slice(fh * F2, (fh + 1) * F2)
                    # consume k in lo/hi interleave (k0, k16, k1, k17...)
                    # matching the two DMA queues carrying each f-half,
                    # so a slow queue half doesn't stall the fill
                    for kk in range(KT // 2):
                        for k in (kk, KT // 2 + kk):
                            nc.tensor.matmul(
                                g_ps[:, fcols],
                                lhsT=xt[:, k, i * P : (i + 1) * P],
                                rhs=w1ts_sb[:, fh, k, :],
                                start=(kk == 0 and k == 0),
                                stop=False,
                            )

                # ---- per t-tile: k-loops, then (corr MM, gelu, fc2)
                # interleaved right behind them so the epilogue of t-tile i
                # hides under t-tile i+1's k-loops and PSUM frees early. ----
                def emit_epilogue(i, g_ps):
                    for fh in range(2):
                        fcols = slice(fh * F2, (fh + 1) * F2)
                        nc.tensor.matmul(
                            g_ps[:, fcols],
                            lhsT=nr[:, i, :],
                            rhs=cs_sb[:, fcols],
                            start=False,
                            stop=(not has_bias1),
                        )
                        if has_bias1:
                            nc.tensor.matmul(
                                g_ps[:, fcols],
                                lhsT=ir[:, i, :],
                                rhs=b1r_sb[:, fcols],
                                start=False,
                                stop=True,
                            )
                    g_sb = gpool.tile([P, F], F16, tag="g_sb")
                    nc.scalar.activation(
                        out=g_sb, in_=g_ps, func=AF.Gelu, scale=y[:, i : i + 1]
                    )
                    fc2s = fc2scr_pool.tile([P, F], F16, tag="fc2s")
                    gi = c * TT + i
                    nc.vector.tensor_tensor(
                        out=fc2s, in0=g_sb, in1=w2b_sb, op=ALU.mult
                    )
                    nc.vector.reduce_sum(
                        outcols[:, gi : gi + 1], fc2s, axis=mybir.AxisListType.X
                    )

                psums = []
                for i in range(TT):
                    g_ps = psum_pool.tile([P, F], F32, tag="g_ps")
                    for fh in range(2):
                        emit_group(i, fh, g_ps)
                    if c == 0:
                        psums.append(g_ps)
                    else:
                        emit_epilogue(i, g_ps)

                if c == 0:
                    # chunk 0: stats only land ~22us in, so the rows and
                    # epilogues all go after the k-loops
                    nr, ir = emit_nmr_rows()
                    for i in range(TT):
                        emit_epilogue(i, psums[i])

                if c + 2 < N_CHUNKS:
                    nxt = load_chunk(c + 2)

            if bias2_val != 0.0:
                nc.vector.tensor_scalar_add(outcols, outcols, bias2_val)
            # transpose [128, 16] -> [16, 128] on the PE so the out DMA is
            # 16 contiguous 512B rows instead of 2048 4-byte descriptors
            otp = opsum_pool.tile([N_TTILES, P], F32, tag="otp")
            nc.tensor.transpose(otp[:], outcols[:], ident32[:])
            nc.vector.tensor_copy(out=outrow, in_=otp)
            nc.sync.dma_start(out=out_d, in_=outrow)

    nc.compile()
    return nc


def _prep_host(hidden_states, ln_gamma, ln_beta, w1, bias1, w2, bias2):
    """Host-side marshalling: dtype casts, layout transposes, exact (fp64)
    folding of the LN affine params into fc1."""
    g64 = np.asarray(ln_gamma, np.float64)
    b64 = np.asarray(ln_beta, np.float64)
    w1_64 = np.asarray(w1, np.float64)
    w1g = np.ascontiguousarray((w1_64 * g64[None, :]).T).astype(np.float16)
    # [4096, 1024] -> [128, 2, 32, 512]: w1ts[p, fh, k, j] = w1g[k*128+p, fh*512+j]
    w1ts = np.ascontiguousarray(
        w1g.reshape(KT, P, 2, F2).transpose(1, 2, 0, 3)
    )
    # colsum of the fp16-quantized folded weights (consistency with device MM)
    cs = np.ascontiguousarray(
        w1g.astype(np.float64).sum(axis=0).reshape(1, F)
    ).astype(np.float16)
    b1_eff = (np.asarray(bias1, np.float64) + w1_64 @ b64).astype(np.float32)
    b1r = b1_eff.reshape(1, F).astype(np.float16)
    w2b = np.broadcast_to(
        np.asarray(w2, np.float64).reshape(1, F).astype(np.float16), (P, F)
    ).copy()
    bias2_val = float(np.asarray(bias2).reshape(-1)[0])
    x2 = np.ascontiguousarray(
        np.asarray(hidden_states, np.float32).reshape(T_TOT, H)
    ).astype(np.float16)
    return x2, w1ts, cs, b1r, w2b, bias2_val


_CACHE = {}


def _get_program(has_bias1, bias2_val):
    key = (has_bias1, bias2_val)
    if key not in _CACHE:
        _CACHE[key] = build_program(has_bias1, bias2_val)
    return _CACHE[key]


def make_in_maps(inputs):
    x2, w1ts, cs, b1r, w2b, bias2_val = _prep_host(**inputs)
    has_bias1 = bool(np.any(np.asarray(b1r) != 0.0))
    in_maps = []
    import ml_dtypes

    for core in range(N_CORES):
        xc = x2[core * T_CORE : (core + 1) * T_CORE]  # [2048, 4096]
        # xts[p, c, k, t] = xc[c*256+t, k*128+p]
        xts = np.ascontiguousarray(
            xc.reshape(N_CHUNKS, CHUNK_T, KT, P).transpose(3, 0, 2, 1)
        )
        m = {
            "xts": xts,
            "xs": np.ascontiguousarray(xc).astype(ml_dtypes.float8_e4m3),
            "w1ts": w1ts,
            "cs": cs,
            "w2b": w2b,
        }
        if has_bias1:
            m["b1r"] = b1r
        in_maps.append(m)
    return in_maps, has_bias1, bias2_val


def kernel(**inputs) -> np.ndarray:
    in_maps, has_bias1, bias2_val = make_in_maps(inputs)
    nc = _get_program(has_bias1, bias2_val)
    res = run_bass_kernel_spmd(nc, in_maps, core_ids=list(range(N_CORES)))
    out = np.concatenate(
        [np.asarray(res.results[i]["out"]).reshape(-1) for i in range(N_CORES)]
    )
    return out.reshape(B, L).astype(np.float32)



# revision 13
# speedup vs baseline: 1.0415x; 1.0415x over previous
"""Trainium2 Bass kernel for nn_AcceptHead: out = fc2(gelu(fc1(LN(x)))).

Self-contained: hardcodes shapes (B=4, L=4096, H=4096, F=1024) and the
data-parallel sharding (tokens split 8 ways, head params replicated).

"W-tilde" architecture: the device PE stream is *only* fc1 matmuls.
LayerNorm is folded into the WEIGHTS and the matmul epilogue:

    LN(x)@W = r_t * (x @ W~)        where W~ = (gamma.*W) column-centered

  The mean term vanishes because mu_t*colsum(W) is itself linear in x:
  x @ (ones*colsum/H) = mu_t*colsum, so subtracting the per-column mean
  of the gamma-folded weights absorbs the -mu correction exactly.
  (Done in fp64 on host; param-only preprocessing like the gamma fold.)

  - x arrives already TRANSPOSED from the host (xts: [128, chunk, k, tok]
    fp16, 256-token chunk-blocked) -- no on-device transpose/normalize.
  - fc1: lhsT = xT block [128h x 128t], rhs = w1ts [128h x 512f], PSUM
    accumulation over 32 k-tiles (fp16 -- the PE floor, ~221us @2.4GHz
    at the measured 518cyc back-to-back N=512 issue rate).
  - rstd r_t is applied as the per-partition `scale` AP of the Gelu
    activation: g = Gelu(r * psum). Newton rsqrt on DVE (bit-trick seed).
  - stats (sum + sumsq, both on DVE: reduce_sum + scalar_tensor_tensor
    square with accum_out) come from a second wire copy of x in [tok, H]
    fp8 layout; stats are emitted a chunk ahead of their epilogue.
  - fc2 is ONE fused DVE op: (g * w2_bcast) with accum_out -> out column.
  - out written [tile, 128] via a final PE transpose so the out DMA is 16
    contiguous rows.

Schedule (all load-bearing; DMA is shared ~350 GB/s/core across queues):
  - phase 0 = chunks 0+1 processed K-MAJOR across all 4 token-tiles, so
    each w1 block's first touch feeds 8 matmuls (~222 GB/s demand instead
    of ~540 for tile-major first touch, which starved the PE for ~20us).
  - fill uses 5 DMA rings in exact consumption order with ramped block
    sizes (first blocks small so the first matmul starts ~1us in):
      SP: w1 fh0 lo-k | SWDGE: w1 fh0 hi-k, then xs c0/c1, w2b
      DVE: w1 fh1 lo-k | PE ring: w1 fh1 hi-k | ACT: xt0+xt1 interleaved
    k is consumed lo/hi-interleaved (k0,k16,k1,...) matching the lo/hi
    ring split so a slow ring half doesn't stall the chase.
  - chunks 2..7 are tile-major (w1 resident; per-tile epilogue hides
    under the next tile's k-loops; PSUM bufs rotate 2-in-flight).
  - chunks 2,3 x-loads are queued on SP behind fh0-lo during the fill;
    per-chunk bodies prefetch chunk c+2 and emit stats for c+1.
  - PSUM: 4 bufs x [128,1024]f32 = all 8 banks; the final out-transpose
    reuses a rotated slot.

fp8 matmul was investigated and rejected: DoubleRow measures 2x fp16 per
unit contraction on this HW (3826ns vs 7386ns per K=4096,N=512 group),
and at 2x every precision-passing fp8 scheme costs the same as fp16
(1-pass e4m3 fails the 2e-2 gate at 4.2e-2 measured in simulation).
"""

import os
import sys

for _p in ("/opt/trn_rl_repo", "/root/.axon_site/_ro/trn_rl_repo"):
    if os.path.isdir(_p) and _p not in sys.path:
        sys.path.append(_p)

import numpy as np

import concourse.bacc as bacc
import concourse.mybir as mybir
import concourse.tile as tile
from concourse.bass_utils import run_bass_kernel_spmd

N_CORES = 8
B, L, H = 4, 4096, 4096
F = H // 4
F2 = F // 2                   # 512, f-half width
T_TOT = B * L                 # 16384 tokens
T_CORE = T_TOT // N_CORES     # 2048 tokens per core
P = 128
KT = H // P                   # 32 contraction tiles
CHUNK_T = 256                 # tokens per pipeline chunk
N_CHUNKS = T_CORE // CHUNK_T  # 8
TT = CHUNK_T // P             # t-tiles per chunk (2)
N_TTILES = T_CORE // P        # 16
EPS = 1e-5
RSQRT_MAGIC = 0x5F3759DF

F16 = mybir.dt.float16
F32 = mybir.dt.float32
F8 = mybir.dt.float8e4
I32 = mybir.dt.int32
AF = mybir.ActivationFunctionType
ALU = mybir.AluOpType

# ramped DMA block sizes (k-tiles) within each 16-tile half: small first
# blocks so the PE's first matmul waits ~1us, bigger ones for efficiency
RAMP = [(0, 1), (1, 2), (2, 4), (4, 8), (8, 12), (12, 16)]


def build_program(has_bias1: bool, bias2_val: float):
    nc = bacc.Bacc(
        "TRN2",
        target_bir_lowering=False,
        debug=False,
        enable_asserts=False,
        num_devices=N_CORES,
    )
    # x, transposed+chunk-blocked on host: xts[p, c, k, t] = x[c*256+t, k*128+p]
    xts_d = nc.dram_tensor(
        "xts", [P, N_CHUNKS, KT, CHUNK_T], F16, kind="ExternalInput"
    ).ap()
    # x, natural [tok, H] layout (stats only; fp8 halves its wire cost and
    # the ~2.6% quantization only perturbs mu/r by ~4e-4 relative)
    xs_d = nc.dram_tensor("xs", [T_CORE, H], F8, kind="ExternalInput").ap()
    # w1 (gamma-folded, column-centered, transposed):
    #   w1ts[p, fh, k, j] = w1c[k*128+p, fh*512+j]
    w1ts_d = nc.dram_tensor(
        "w1ts", [P, 2, KT, F2], F16, kind="ExternalInput"
    ).ap()
    w2b_d = nc.dram_tensor("w2b", [P, F], F16, kind="ExternalInput").ap()
    if has_bias1:
        b1b_d = nc.dram_tensor("b1b", [P, F], F32, kind="ExternalInput").ap()
    # out as [t-tile, partition]: token t = n*128+p lives at out[n, p], so
    # the flattened DRAM tensor IS token order.
    out_d = nc.dram_tensor(
        "out", [N_TTILES, P], F32, kind="ExternalOutput"
    ).ap()

    with tile.TileContext(nc) as tc:
        with (
            tc.tile_pool(name="singles", bufs=1) as singles,
            tc.tile_pool(name="xtpool", bufs=4) as xtpool,
            tc.tile_pool(name="xspool", bufs=8) as xspool,
            tc.tile_pool(name="sqscr", bufs=2) as sqscr_pool,
            tc.tile_pool(name="gpool", bufs=2) as gpool,
            tc.tile_pool(name="fc2scr", bufs=2) as fc2scr_pool,
            tc.tile_pool(name="stats", bufs=4) as stats,
            tc.tile_pool(name="psum", bufs=4, space="PSUM") as psum_pool,
        ):
            w1ts_sb = singles.tile([P, 2, KT, F2], F16)
            w2b_sb = singles.tile([P, F], F16)
            if has_bias1:
                b1b_sb = singles.tile([P, F], F32)
            # outcols padded to 32 free cols for the DVE block-transpose
            outcols = singles.tile([P, 32], F32)
            vt = singles.tile([P, 32], F32)
            nc.vector.memset(outcols[:, N_TTILES:], 0.0)

            # ---- fill-phase DMA schedule (3 rings; see docstring) ----
            def w1_dma(eng, fh, ka, kb):
                eng.dma_start(
                    out=w1ts_sb[:, fh, ka:kb, :], in_=w1ts_d[:, fh, ka:kb, :]
                )

            for a, b in RAMP:                       # SP: lo-k, fh pairs
                w1_dma(nc.sync, 0, a, b)
                w1_dma(nc.sync, 1, a, b)
            for a, b in RAMP:                       # SWDGE: hi-k, fh pairs
                w1_dma(nc.gpsimd, 0, 16 + a, 16 + b)
                w1_dma(nc.gpsimd, 1, 16 + a, 16 + b)
            # ACT ring: xt0 + xt1 interleaved in the k consumption order
            xt0 = xtpool.tile([P, KT, CHUNK_T], F16, tag="xt")
            xt1 = xtpool.tile([P, KT, CHUNK_T], F16, tag="xt")
            for a, b in RAMP:
                for lo in (0, 16):
                    nc.scalar.dma_start(
                        out=xt0[:, lo + a : lo + b, :],
                        in_=xts_d[:, 0, lo + a : lo + b, :],
                    )
                    nc.scalar.dma_start(
                        out=xt1[:, lo + a : lo + b, :],
                        in_=xts_d[:, 1, lo + a : lo + b, :],
                    )

            def load_xs(c, eng):
                xss = []
                for i in range(TT):
                    xsb = xspool.tile([P, H], F8, tag="xs")
                    row0 = c * CHUNK_T + i * P
                    eng.dma_start(out=xsb, in_=xs_d[row0 : row0 + P, :])
                    xss.append(xsb)
                return xss

            def load_chunk(c, eng):
                xt = xtpool.tile([P, KT, CHUNK_T], F16, tag="xt")
                for k0 in range(0, KT, 8):
                    eng.dma_start(
                        out=xt[:, k0 : k0 + 8, :], in_=xts_d[:, c, k0 : k0 + 8, :]
                    )
                return xt, load_xs(c, eng)

            # behind the w1/xt streams: xs0 on SP, xs1 on ACT, chunk-2
            # inputs on SP, chunk-3 inputs on ACT, small params on SWDGE
            xss0 = load_xs(0, nc.sync)
            xss1 = load_xs(1, nc.scalar)
            nc.gpsimd.dma_start(out=w2b_sb, in_=w2b_d)
            if has_bias1:
                nc.gpsimd.dma_start(out=b1b_sb, in_=b1b_d)

            # ---- stats chain (all DVE), emitted a chunk ahead of use ----
            def emit_stats(xss):
                sums = stats.tile([P, TT], F32, tag="sums")
                sq = stats.tile([P, TT], F32, tag="sq")
                for i in range(TT):
                    nc.vector.reduce_sum(
                        sums[:, i : i + 1], xss[i], axis=mybir.AxisListType.X
                    )
                    sqs = sqscr_pool.tile([P, H], F16, tag="sqs")
                    nc.scalar.activation(
                        out=sqs, in_=xss[i], func=AF.Square,
                        accum_out=sq[:, i : i + 1],
                    )
                mu = stats.tile([P, TT], F32, tag="mu")
                nc.vector.tensor_scalar_mul(mu, sums, 1.0 / H)
                vv = stats.tile([P, TT], F32, tag="vv")
                # vv = sq/H - mu^2 + eps
                nc.vector.tensor_tensor(out=vv, in0=mu, in1=mu, op=ALU.mult)
                nc.vector.tensor_scalar(
                    out=vv, in0=vv, scalar1=-1.0, scalar2=EPS,
                    op0=ALU.mult, op1=ALU.add,
                )
                nc.vector.tensor_scalar(
                    out=sq, in0=sq, scalar1=1.0 / H, scalar2=None, op0=ALU.mult
                )
                nc.vector.tensor_tensor(out=vv, in0=vv, in1=sq, op=ALU.add)
                # Newton rsqrt: y0 via bit trick, 2 iterations
                y = stats.tile([P, TT], F32, tag="y")
                yi = y[:].bitcast(I32)
                nc.vector.tensor_scalar(
                    out=yi, in0=vv[:].bitcast(I32), scalar1=1, scalar2=None,
                    op0=ALU.arith_shift_right,
                )
                nc.vector.tensor_scalar(
                    out=yi, in0=yi, scalar1=-1, scalar2=RSQRT_MAGIC,
                    op0=ALU.mult, op1=ALU.add,
                )
                h_half = stats.tile([P, TT], F32, tag="h_half")
                nc.vector.tensor_scalar_mul(h_half, vv, 0.5)
                u = stats.tile([P, TT], F32, tag="u")
                for _ in range(2):
                    nc.vector.tensor_tensor(out=u, in0=y, in1=y, op=ALU.mult)
                    nc.vector.tensor_tensor(out=u, in0=u, in1=h_half, op=ALU.mult)
                    nc.vector.tensor_scalar(
                        out=u, in0=u, scalar1=-1.0, scalar2=1.5,
                        op0=ALU.mult, op1=ALU.add,
                    )
                    nc.vector.tensor_tensor(out=y, in0=y, in1=u, op=ALU.mult)
                return y

            # ---- epilogue: gelu(r*psum) then fused fc2 dot on DVE ----
            def emit_epilogue(gi, g_ps, y, yi):
                g_sb = gpool.tile([P, F], F16, tag="g_sb")
                if has_bias1:
                    pre = gpool.tile([P, F], F16, tag="pre")
                    nc.vector.scalar_tensor_tensor(
                        out=pre, in0=g_ps, scalar=y[:, yi : yi + 1],
                        in1=b1b_sb, op0=ALU.mult, op1=ALU.add,
                    )
                    nc.scalar.activation(out=g_sb, in_=pre, func=AF.Gelu)
                else:
                    nc.scalar.activation(
                        out=g_sb, in_=g_ps, func=AF.Gelu,
                        scale=y[:, yi : yi + 1],
                    )
                fc2s = fc2scr_pool.tile([P, F], F16, tag="fc2s")
                nc.vector.scalar_tensor_tensor(
                    out=fc2s, in0=g_sb, scalar=1.0, in1=w2b_sb,
                    op0=ALU.mult, op1=ALU.mult,
                    accum_out=outcols[:, gi : gi + 1],
                )

            # stats for chunk 0 (wait on the xs semaphores, run mid-fill);
            # chunk-1 stats are emitted between the phase-0 epilogues so
            # the c1 Squares queue on ACT *behind* the slot-freeing gelus
            st = {0: emit_stats(xss0)}
            loads = {2: load_chunk(2, nc.sync), 3: load_chunk(3, nc.scalar)}

            # ---- phase 0: chunks 0+1, K-MAJOR across all 4 t-tiles so
            # each w1 block's first touch feeds 8 matmuls. The last 2
            # kk-steps flip to tile-major so tile 0 finishes ~2.6us early
            # and its gelu frees a PSUM slot before chunk 2 needs it. ----
            g_ps4 = [
                psum_pool.tile([P, F], F32, tag="g_ps", name=f"g_ps{j}")
                for j in range(4)
            ]

            def p0_mm(t4, k):
                xt = (xt0, xt1)[t4 // 2]
                i = t4 % 2
                for fh in range(2):
                    nc.tensor.matmul(
                        g_ps4[t4][:, fh * F2 : (fh + 1) * F2],
                        lhsT=xt[:, k, i * P : (i + 1) * P],
                        rhs=w1ts_sb[:, fh, k, :],
                        start=(k == 0),
                        stop=(k == KT - 1),
                    )

            TAIL = 2
            for kk in range(KT // 2 - TAIL):
                for k in (kk, KT // 2 + kk):    # lo/hi interleave, see rings
                    for t4 in range(4):
                        p0_mm(t4, k)
            for t4 in range(4):
                for kk in range(KT // 2 - TAIL, KT // 2):
                    for k in (kk, KT // 2 + kk):
                        p0_mm(t4, k)
                if t4 == 2:
                    st[1] = emit_stats(xss1)
                emit_epilogue(t4, g_ps4[t4], st[t4 // 2], t4 % 2)
            del st[0], st[1]
            st[2] = emit_stats(loads[2][1])

            # ---- chunks 2..7: tile-major; epilogue of tile i hides under
            # tile i+1's k-loops; stats/loads pipelined ahead ----
            for c in range(2, N_CHUNKS):
                xt, xss = loads.pop(c)
                if c + 2 < N_CHUNKS:
                    loads[c + 2] = load_chunk(c + 2, nc.sync)
                y = st.pop(c)
                for i in range(TT):
                    g_ps = psum_pool.tile([P, F], F32, tag="g_ps")
                    for fh in range(2):
                        for k in range(KT):
                            nc.tensor.matmul(
                                g_ps[:, fh * F2 : (fh + 1) * F2],
                                lhsT=xt[:, k, i * P : (i + 1) * P],
                                rhs=w1ts_sb[:, fh, k, :],
                                start=(k == 0),
                                stop=(k == KT - 1),
                            )
                    emit_epilogue(c * TT + i, g_ps, y, i)
                    # stats for c+1 queue behind tile-0's gelu on ACT so
                    # the Squares never delay the PSUM-freeing gelu
                    if i == 0 and c + 1 < N_CHUNKS and c + 1 not in st:
                        st[c + 1] = emit_stats(loads[c + 1][1])

            if bias2_val != 0.0:
                nc.vector.tensor_scalar_add(
                    outcols[:, :N_TTILES], outcols[:, :N_TTILES], bias2_val
                )
            # DVE 32x32 block-transpose: vt[32*bi + n, pj] = outcols[32*bi
            # + pj, n], then 4 strided DMAs with 128B inner runs -- avoids
            # both a PE transpose (PSUM slot) and 4-byte scatter descriptors
            nc.vector.transpose(vt[:], outcols[:])
            for bi in range(4):
                nc.sync.dma_start(
                    out=out_d[:, 32 * bi : 32 * bi + 32],
                    in_=vt[32 * bi : 32 * bi + N_TTILES, :],
                )

    nc.compile()
    return nc


def _prep_host(hidden_states, ln_gamma, ln_beta, w1, bias1, w2, bias2):
    """Host-side marshalling: dtype casts, layout transposes, exact (fp64)
    folding of the LN affine params into fc1 (gamma row-scale + column
    centering, which absorbs the -mu*colsum correction)."""
    g64 = np.asarray(ln_gamma, np.float64)
    b64 = np.asarray(ln_beta, np.float64)
    w1_64 = np.asarray(w1, np.float64)
    w1g = np.ascontiguousarray((w1_64 * g64[None, :]).T)      # [H, F] fp64
    w1c = w1g - w1g.mean(axis=0, keepdims=True)               # column-center
    # [4096, 1024] -> [128, 2, 32, 512]: w1ts[p, fh, k, j] = w1c[k*128+p, fh*512+j]
    w1ts = np.ascontiguousarray(
        w1c.reshape(KT, P, 2, F2).transpose(1, 2, 0, 3)
    ).astype(np.float16)
    b1_eff = (np.asarray(bias1, np.float64) + w1_64 @ b64).astype(np.float32)
    b1b = np.broadcast_to(b1_eff.reshape(1, F), (P, F)).copy()
    w2b = np.broadcast_to(
        np.asarray(w2, np.float64).reshape(1, F).astype(np.float16), (P, F)
    ).copy()
    bias2_val = float(np.asarray(bias2).reshape(-1)[0])
    x2 = np.ascontiguousarray(
        np.asarray(hidden_states, np.float32).reshape(T_TOT, H)
    ).astype(np.float16)
    return x2, w1ts, b1b, w2b, bias2_val


_CACHE = {}


def _get_program(has_bias1, bias2_val):
    key = (has_bias1, bias2_val)
    if key not in _CACHE:
        _CACHE[key] = build_program(has_bias1, bias2_val)
    return _CACHE[key]


def make_in_maps(inputs):
    x2, w1ts, b1b, w2b, bias2_val = _prep_host(**inputs)
    has_bias1 = bool(np.any(np.asarray(b1b) != 0.0))
    in_maps = []
    import ml_dtypes

    for core in range(N_CORES):
        xc = x2[core * T_CORE : (core + 1) * T_CORE]  # [2048, 4096]
        # xts[p, c, k, t] = xc[c*256+t, k*128+p]
        xts = np.ascontiguousarray(
            xc.reshape(N_CHUNKS, CHUNK_T, KT, P).transpose(3, 0, 2, 1)
        )
        m = {
            "xts": xts,
            "xs": np.ascontiguousarray(xc).astype(ml_dtypes.float8_e4m3),
            "w1ts": w1ts,
            "w2b": w2b,
        }
        if has_bias1:
            m["b1b"] = b1b
        in_maps.append(m)
    return in_maps, has_bias1, bias2_val


def kernel(**inputs) -> np.ndarray:
    in_maps, has_bias1, bias2_val = make_in_maps(inputs)
    nc = _get_program(has_bias1, bias2_val)
    res = run_bass_kernel_spmd(nc, in_maps, core_ids=list(range(N_CORES)))
    out = np.concatenate(
        [np.asarray(res.results[i]["out"]).reshape(-1) for i in range(N_CORES)]
    )
    return out.reshape(B, L).astype(np.float32)
